# revision 9
# baseline (speedup 1.0000x reference)
"""MoE (top-2 of 8 experts) Trainium2 kernel, expert-parallel across 8 NeuronCores.

Strategy (matches the expert-parallel sharding hint):
  - Host computes the router (logits -> top-2 -> softmax) and performs the
    token all-to-all: tokens are gathered per expert, padded to a common
    capacity C, and each core gets one expert's tokens + that expert's
    W1/b1/W2 weights.
  - Each core runs a Bass/Tile kernel computing
        y = gelu_exact(x @ W1 + b1) @ W2
    in bf16 (fp32 PSUM accumulate, ~3e-3 rel err, well under the 2e-2 gate).
  - Host scatter-adds the per-expert outputs back with the routing weights
    and adds sum_k w_k * b2[e_k] (folding b2 into the host combine).

Per-core dataflow (two phases, PE never idles between them):
  Phase A (h = gelu(x @ W1 + b1)): stationary = W1 128x128 blocks streamed
  from HBM, moving = xT token chunks; PSUM [f, 512 tok]; exact GELU +
  per-partition bias b1 fused into one ScalarE activation per tile; h kept
  RESIDENT in SBUF as bf16.
  Phase B (y = h @ W2): W2 fully resident in SBUF (prefetched during phase
  A on the second DMA ring, gated behind the x stream); stationary = h
  blocks [128 f, 128 tok], moving = W2 rows [128 f, 512 d]; each token
  pair's y accumulates over all 32 f-tiles in dedicated PSUM banks, then
  drains (ScalarE+VectorE in parallel -> bf16 -> DMA) while the next pair
  accumulates.

  DMAs are batched (~25 total): every DMA costs ~0.7us of trigger issue on
  its queue and ~130ns/engine of end-of-program semaphore retire, which was
  ~13us of pure tail with per-tile transfers.
"""

import numpy as np
import ml_dtypes

import concourse.bass as bass
import concourse.mybir as mybir
import concourse.tile as tile
from concourse import bacc
from concourse.bass_utils import run_bass_kernel_spmd

P = 128
D = 1024
F = 4096
E = 8
TOP_K = 2
DK = D // P   # 8 contraction tiles for GEMM1
FT = F // P   # 32 f tiles
N_CORES = 8
NSOLO = 8     # leading W1 f-tiles shipped as single DMAs (startup latency)
NWARM = 6     # f-tiles run chunk-0-first while the rest of x streams

BF16 = ml_dtypes.bfloat16

_F32 = mybir.dt.float32
_BF16 = mybir.dt.bfloat16

_compiled = {}  # C -> Bacc program


def _token_chunks(C):
    """Split C into 512-token chunks (PSUM-bank-width moving dim)."""
    chunks = []
    off = 0
    while off < C:
        cn = min(512, C - off)
        chunks.append((off, cn))
        off += cn
    return chunks


def _build(C):
    assert C % 256 == 0
    TT = C // P   # token tiles for GEMM2
    chunks = _token_chunks(C)
    NCH = len(chunks)
    NG = (FT - NSOLO) // 4  # W1 4-tile DMA groups
    nc = bacc.Bacc(None, target_bir_lowering=False)

    # x is chunk-major: [chunk][dk][token-within-chunk] per partition row.
    xt_d = nc.dram_tensor("xt", [P, DK * C], _BF16, kind="ExternalInput")
    w1a_d = nc.dram_tensor("w1a", [NSOLO, P, DK, P], _BF16, kind="ExternalInput")
    w1g_d = nc.dram_tensor("w1g", [NG, P, 4, DK, P], _BF16, kind="ExternalInput")
    w2_d = nc.dram_tensor("w2", [4, P, FT // 4, D], _BF16, kind="ExternalInput")
    b1_d = nc.dram_tensor("b1", [P, FT], _F32, kind="ExternalInput")
    y_d = nc.dram_tensor("y", [TT // 2, P, 2, D], _BF16, kind="ExternalOutput")

    with tile.TileContext(nc) as tc:
        with (
            tc.tile_pool(name="xpool", bufs=1) as xpool,
            tc.tile_pool(name="cpool", bufs=1) as cpool,
            tc.tile_pool(name="w1pool", bufs=1) as w1pool,
            tc.tile_pool(name="w2pool", bufs=1) as w2pool,
            tc.tile_pool(name="hpool", bufs=1) as hpool,
            tc.tile_pool(name="ypool", bufs=2) as ypool,
            tc.tile_pool(name="hpsum", bufs=2, space="PSUM") as hpsum,
            tc.tile_pool(name="ypsum", bufs=3, space="PSUM") as ypsum,
        ):
            # x / W1 / b1 ride the sync DMA ring in demand order. W2 rides
            # the gpsimd ring, gated behind the last x chunk (below), so it
            # cannot front-run the startup-critical stream.
            def w1_solo_dma(ft):
                t = w1pool.tile(
                    [P, DK, P], _BF16, tag="w1t", bufs=NSOLO, name=f"w1s{ft}"
                )
                nc.sync.dma_start(out=t[:], in_=w1a_d[ft])
                return t

            def w1_group_dma(g):
                t = w1pool.tile([P, 4, DK, P], _BF16, tag="w1g", bufs=3, name=f"w1g{g}")
                nc.sync.dma_start(out=t[:], in_=w1g_d[g])
                return t

            w2_sb = w2pool.tile([P, FT // 4, 4, D], _BF16, name="w2sb")
            h_sb = [
                hpool.tile([P, FT, cn], _BF16, tag=f"hc{ci}", name=f"hc{ci}")
                for ci, (_, cn) in enumerate(chunks)
            ]

            # Startup demand order on the sync ring.
            xt_sb = []
            solo = {}
            for ci, (c0, cn) in enumerate(chunks):
                t = xpool.tile([P, DK, cn], _BF16, tag=f"xt{ci}", name=f"xt{ci}")
                o = c0 * DK
                nc.sync.dma_start(out=t[:], in_=xt_d[:, o : o + DK * cn])
                xt_sb.append(t)
                if ci == 0:
                    solo[0] = w1_solo_dma(0)
                    b1_sb = cpool.tile([P, FT], _F32)
                    nc.sync.dma_start(out=b1_sb[:], in_=b1_d[:])
                    for ft in range(1, 4):
                        solo[ft] = w1_solo_dma(ft)
            for ft in range(4, NSOLO):
                solo[ft] = w1_solo_dma(ft)
            groups = {0: w1_group_dma(0), 1: w1_group_dma(1)}

            # W2 prefetch on the gpsimd ring, gated behind the x stream.
            gate = cpool.tile([P, 4], _BF16, tag="gate")
            nc.gpsimd.tensor_copy(gate[:], xt_sb[-1][:, 0, 0:4])
            for g in range(4):
                nc.gpsimd.dma_start(out=w2_sb[:, :, g, :], in_=w2_d[g])

            # PE warm-up: dummy zero matmuls with no DMA deps run during the
            # initial input-DMA wait, so the HAM clock gate reaches 2.4 GHz
            # before the real stream starts.
            warm = cpool.tile([P, 512], _BF16, tag="warm")
            nc.gpsimd.memset(warm[:], 0.0)
            for r in range(2):
                pw = hpsum.tile([P, 512], _F32, tag="ph", name=f"pw{r}")
                for k in range(4):
                    nc.tensor.matmul(
                        pw[:], warm[:, :P], warm[:], start=(k == 0), stop=(k == 3)
                    )

            def w1_block(ft):
                # (tile, index fn) for f-tile ft
                if ft < NSOLO:
                    return solo[ft], None
                return groups[(ft - NSOLO) // 4], (ft - NSOLO) % 4

            def gemm1_group(ft, ci):
                c0, cn = chunks[ci]
                w1t, j = w1_block(ft)
                ph = hpsum.tile([P, 512], _F32, tag="ph")
                for dk in range(DK):
                    st = w1t[:, dk, :] if j is None else w1t[:, j, dk, :]
                    nc.tensor.matmul(
                        ph[:, :cn],
                        st,
                        xt_sb[ci][:, dk, :],
                        start=(dk == 0),
                        stop=(dk == DK - 1),
                    )
                nc.scalar.activation(
                    h_sb[ci][:, ft, :],
                    ph[:, :cn],
                    mybir.ActivationFunctionType.Gelu,
                    bias=b1_sb[:, ft : ft + 1],
                    scale=1.0,
                )

            # Phase A. The first NWARM f-tiles run chunk 0 only, so the PE
            # has work while the rest of x is still in flight.
            order = [(ft, 0) for ft in range(NWARM)]
            order += [(ft, ci) for ci in range(1, NCH) for ft in range(NWARM)]
            order += [(ft, ci) for ft in range(NWARM, FT) for ci in range(NCH)]
            for ft, ci in order:
                if ci == 0 and ft >= NSOLO and ft % 4 == 0:
                    g = (ft - NSOLO) // 4 + 2
                    if g < NG and g not in groups:
                        groups[g] = w1_group_dma(g)
                gemm1_group(ft, ci)

            # Phase B: token pairs, full 32-step PSUM accumulation per pair.
            for tq in range(TT // 2):
                ci = (tq * 2 * P) // 512  # chunk holding this token pair
                cb = tq * 2 * P - chunks[ci][0]  # base token within chunk
                accs = [
                    ypsum.tile([P, D], _F32, tag="py", name=f"py{tq}_{i}")
                    for i in range(2)
                ]
                for ft in range(FT):
                    for tt2 in range(2):
                        hblk = h_sb[ci][:, ft, cb + tt2 * P : cb + (tt2 + 1) * P]
                        for dh in range(2):
                            nc.tensor.matmul(
                                accs[tt2][:, dh * 512 : (dh + 1) * 512],
                                hblk,
                                w2_sb[:, ft // 4, ft % 4, dh * 512 : (dh + 1) * 512],
                                start=(ft == 0),
                                stop=(ft == FT - 1),
                            )
                ysb = ypool.tile([P, 2, D], _BF16, tag="ysb")
                for tt2 in range(2):
                    # Drain the two PSUM banks in parallel on Scalar+Vector.
                    nc.scalar.activation(
                        ysb[:, tt2, :512],
                        accs[tt2][:, :512],
                        mybir.ActivationFunctionType.Copy,
                    )
                    nc.vector.tensor_copy(ysb[:, tt2, 512:], accs[tt2][:, 512:])
                eng = nc.sync if tq % 2 == 0 else nc.gpsimd
                eng.dma_start(out=y_d[tq], in_=ysb[:])

    nc.compile()
    return nc


def _route(xf, Wr, br):
    """Host router: exact top-2 + softmax weights (float64 for stable order)."""
    logits = xf.astype(np.float64) @ Wr.astype(np.float64) + br.astype(np.float64)
    order = np.argsort(-logits, axis=1, kind="stable")
    top2 = order[:, :TOP_K]  # [T, 2]
    v = np.take_along_axis(logits, top2, axis=1)
    v = v - v.max(axis=1, keepdims=True)
    ev = np.exp(v)
    rw = (ev / ev.sum(axis=1, keepdims=True)).astype(np.float32)  # [T, 2]
    return top2, rw


def _run(x, Wr, br, W1, b1, W2, b2, trace=False):
    B, S, d = x.shape
    T = B * S
    xf = np.ascontiguousarray(np.asarray(x, dtype=np.float32).reshape(T, d))

    top2, rw = _route(xf, Wr, br)

    token_lists = []
    weight_lists = []
    for e in range(E):
        in_slot0 = top2[:, 0] == e
        in_slot1 = top2[:, 1] == e
        toks = np.nonzero(in_slot0 | in_slot1)[0]
        w = np.where(in_slot0[toks], rw[toks, 0], rw[toks, 1]).astype(np.float32)
        token_lists.append(toks)
        weight_lists.append(w)

    # Capacity: balanced mean (rounded up to 256), capped by the SBUF
    # working set (x + h + W2 are resident). Pairs beyond it are computed
    # on the host - cheap for near-balanced routing.
    C = max(256, min(1024, -(-(T * TOP_K // E) // 256) * 256))
    spill_lists = [(t[C:], w[C:]) for t, w in zip(token_lists, weight_lists)]
    token_lists = [t[:C] for t in token_lists]
    weight_lists = [w[:C] for w in weight_lists]

    if C not in _compiled:
        _compiled[C] = _build(C)
    nc = _compiled[C]

    # Per-expert weight layouts (see _build DRAM shapes)
    W1 = np.asarray(W1, dtype=np.float32)
    W2 = np.asarray(W2, dtype=np.float32)
    b1 = np.asarray(b1, dtype=np.float32)
    b2 = np.asarray(b2, dtype=np.float32)
    w1h = np.ascontiguousarray(
        W1.reshape(E, DK, P, FT, P).transpose(0, 3, 2, 1, 4)
    ).astype(BF16)  # [E, FT, P(dp), DK, P(fi)]
    w1a = np.ascontiguousarray(w1h[:, :NSOLO])  # [E, NSOLO, P, DK, P]
    NG = (FT - NSOLO) // 4
    w1g = np.ascontiguousarray(
        w1h[:, NSOLO:].reshape(E, NG, 4, P, DK, P).transpose(0, 1, 3, 2, 4, 5)
    )  # [E, NG, P, 4, DK, P]
    w2h = np.ascontiguousarray(
        W2.reshape(E, FT // 4, 4, P, D).transpose(0, 2, 3, 1, 4)
    ).astype(BF16)  # [E, 4(g), P(fp), FT//4, D]
    b1h = np.ascontiguousarray(b1.reshape(E, FT, P).transpose(0, 2, 1))  # [E, P, FT]

    in_maps = []
    for e in range(E):
        toks = token_lists[e]
        xg = np.zeros((C, d), dtype=np.float32)
        xg[: len(toks)] = xf[toks]
        xt = np.empty((P, DK * C), dtype=BF16)
        for c0, cn in _token_chunks(C):
            blk = xg[c0 : c0 + cn].T.reshape(DK, P, cn).transpose(1, 0, 2)
            xt[:, c0 * DK : c0 * DK + DK * cn] = blk.reshape(P, DK * cn).astype(BF16)
        in_maps.append(
            {"xt": xt, "w1a": w1a[e], "w1g": w1g[e], "w2": w2h[e], "b1": b1h[e]}
        )

    res = run_bass_kernel_spmd(
        nc, in_maps, core_ids=list(range(N_CORES)), trace=trace
    )

    # Host combine: out[t] = sum_k rw[t,k] * (y_{e_k}(t) + b2[e_k])
    w_dense = np.zeros((T, E), dtype=np.float32)
    np.put_along_axis(w_dense, top2, rw, axis=1)
    out = w_dense @ b2  # [T, D] bias part
    for e in range(E):
        toks = token_lists[e]
        yr = np.asarray(res.results[e]["y"], dtype=np.float32)  # [TT//2, P, 2, D]
        y = yr.transpose(0, 2, 1, 3).reshape(C, d)
        out[toks] += weight_lists[e][:, None] * y[: len(toks)]

    # Host-side spill: overflow pairs beyond the device capacity.
    try:
        from scipy.special import erf
    except ImportError:
        import math

        erf = np.vectorize(math.erf, otypes=[np.float32])

    sqrt2 = np.float32(np.sqrt(2.0))
    for e in range(E):
        toks, w = spill_lists[e]
        if len(toks) == 0:
            continue
        hs = xf[toks] @ W1[e] + b1[e]
        hs = 0.5 * hs * (1.0 + erf(hs / sqrt2))
        ys = hs @ W2[e]
        out[toks] += w[:, None] * ys

    return out.reshape(B, S, d).astype(np.float32), res


def kernel(x, Wr, br, W1, b1, W2, b2):
    out, _ = _run(x, Wr, br, W1, b1, W2, b2, trace=False)
    return out


# revision 11
# speedup vs baseline: 1.0796x; 1.0796x over previous
"""MoE (top-2 of 8 experts) Trainium2 kernel, expert-parallel across 8 NeuronCores.

Strategy (matches the expert-parallel sharding hint):
  - Host computes the router (logits -> top-2 -> softmax) and performs the
    token all-to-all: tokens are gathered per expert, padded to a common
    capacity C, and each core gets one expert's tokens + that expert's
    W1/b1/W2 weights.
  - Each core runs a Bass/Tile kernel computing
        y = gelu_exact(x @ W1 + b1) @ W2
    in bf16 (fp32 PSUM accumulate, ~3e-3 rel err, well under the 2e-2 gate).
  - Host scatter-adds the per-expert outputs back with the routing weights
    and adds sum_k w_k * b2[e_k] (folding b2 into the host combine).

Per-core dataflow (two phases, PE never idles between them):
  Phase A (h = gelu(x @ W1 + b1)): stationary = W1 128x128 blocks streamed
  from HBM, moving = xT token chunks; PSUM [f, 512 tok]; exact GELU +
  per-partition bias b1 fused into one ScalarE activation per tile; h kept
  RESIDENT in SBUF as bf16.
  Phase B (y = h @ W2): W2 fully resident in SBUF (prefetched during phase
  A on the second DMA ring, gated behind the x stream); stationary = h
  blocks [128 f, 128 tok], moving = W2 rows [128 f, 512 d]; each token
  pair's y accumulates over all 32 f-tiles in dedicated PSUM banks, then
  drains (ScalarE+VectorE in parallel -> bf16 -> DMA) while the next pair
  accumulates.

  DMAs are batched (~25 total): every DMA costs ~0.7us of trigger issue on
  its queue and ~130ns/engine of end-of-program semaphore retire, which was
  ~13us of pure tail with per-tile transfers.
"""

import numpy as np
import ml_dtypes

import concourse.bass as bass
import concourse.mybir as mybir
import concourse.tile as tile
from concourse import bacc
from concourse.bass_utils import run_bass_kernel_spmd

P = 128
D = 1024
F = 4096
E = 8
TOP_K = 2
DK = D // P   # 8 contraction tiles for GEMM1
FT = F // P   # 32 f tiles
N_CORES = 8
NSOLO = 8     # leading W1 f-tiles shipped as single DMAs (startup latency)
NWARM = 6     # f-tiles run chunk-0-first while the rest of x streams

BF16 = ml_dtypes.bfloat16

_F32 = mybir.dt.float32
_BF16 = mybir.dt.bfloat16

_compiled = {}  # C -> Bacc program


def _token_chunks(C):
    """Split C into 512-token chunks (PSUM-bank-width moving dim)."""
    chunks = []
    off = 0
    while off < C:
        cn = min(512, C - off)
        chunks.append((off, cn))
        off += cn
    return chunks


def _build(C):
    assert C % 256 == 0
    TT = C // P   # token tiles for GEMM2
    chunks = _token_chunks(C)
    NCH = len(chunks)
    NG = (FT - NSOLO) // 4  # W1 4-tile DMA groups
    nc = bacc.Bacc(None, target_bir_lowering=False)

    # x is chunk-major: [chunk][dk][token-within-chunk] per partition row.
    xt_d = nc.dram_tensor("xt", [P, DK * C], _BF16, kind="ExternalInput")
    w1a_d = nc.dram_tensor("w1a", [NSOLO, P, DK, P], _BF16, kind="ExternalInput")
    w1g_d = nc.dram_tensor("w1g", [NG, P, 4, DK, P], _BF16, kind="ExternalInput")
    w2_d = nc.dram_tensor("w2", [4, P, FT // 4, D], _BF16, kind="ExternalInput")
    b1_d = nc.dram_tensor("b1", [P, FT], _F32, kind="ExternalInput")
    y_d = nc.dram_tensor("y", [TT // 2, P, 2, D], _BF16, kind="ExternalOutput")

    with tile.TileContext(nc) as tc:
        with (
            tc.tile_pool(name="xpool", bufs=1) as xpool,
            tc.tile_pool(name="cpool", bufs=1) as cpool,
            tc.tile_pool(name="w1pool", bufs=1) as w1pool,
            tc.tile_pool(name="w2pool", bufs=1) as w2pool,
            tc.tile_pool(name="hpool", bufs=1) as hpool,
            tc.tile_pool(name="ypool", bufs=2) as ypool,
            tc.tile_pool(name="hpsum", bufs=2, space="PSUM") as hpsum,
            tc.tile_pool(name="ypsum", bufs=3, space="PSUM") as ypsum,
        ):
            # x / W1 / b1 ride the sync DMA ring in demand order. W2 rides
            # the gpsimd ring, gated behind the last x chunk (below), so it
            # cannot front-run the startup-critical stream.
            def w1_solo_dma(ft):
                t = w1pool.tile(
                    [P, DK, P], _BF16, tag="w1t", bufs=NSOLO, name=f"w1s{ft}"
                )
                nc.sync.dma_start(out=t[:], in_=w1a_d[ft])
                return t

            def w1_group_dma(g):
                t = w1pool.tile([P, 4, DK, P], _BF16, tag="w1g", bufs=3, name=f"w1g{g}")
                nc.sync.dma_start(out=t[:], in_=w1g_d[g])
                return t

            w2_sb = w2pool.tile([P, FT // 4, 4, D], _BF16, name="w2sb")
            h_sb = [
                hpool.tile([P, FT, cn], _BF16, tag=f"hc{ci}", name=f"hc{ci}")
                for ci, (_, cn) in enumerate(chunks)
            ]

            # Startup demand order on the sync ring.
            xt_sb = []
            solo = {}
            for ci, (c0, cn) in enumerate(chunks):
                t = xpool.tile([P, DK, cn], _BF16, tag=f"xt{ci}", name=f"xt{ci}")
                o = c0 * DK
                nc.sync.dma_start(out=t[:], in_=xt_d[:, o : o + DK * cn])
                xt_sb.append(t)
                if ci == 0:
                    solo[0] = w1_solo_dma(0)
                    b1_sb = cpool.tile([P, FT], _F32)
                    nc.sync.dma_start(out=b1_sb[:], in_=b1_d[:])
                    for ft in range(1, 4):
                        solo[ft] = w1_solo_dma(ft)
            for ft in range(4, NSOLO):
                solo[ft] = w1_solo_dma(ft)
            groups = {0: w1_group_dma(0), 1: w1_group_dma(1)}

            # PE warm-up: dummy zero matmuls with no DMA deps run during the
            # initial input-DMA wait, so the HAM clock gate reaches 2.4 GHz
            # before the real stream starts. The memset rides VectorE, whose
            # queue has nothing else early, so the warm-up starts ~6us in.
            warm = cpool.tile([P, 512], _BF16, tag="warm")
            nc.vector.memset(warm[:], 0.0)
            for r in range(2):
                pw = hpsum.tile([P, 512], _F32, tag="ph", name=f"pw{r}")
                for k in range(4):
                    nc.tensor.matmul(
                        pw[:], warm[:, :P], warm[:], start=(k == 0), stop=(k == 3)
                    )

            def w1_block(ft):
                # (tile, index fn) for f-tile ft
                if ft < NSOLO:
                    return solo[ft], None
                return groups[(ft - NSOLO) // 4], (ft - NSOLO) % 4

            def gemm1_group(ft, ci):
                c0, cn = chunks[ci]
                w1t, j = w1_block(ft)
                ph = hpsum.tile([P, 512], _F32, tag="ph")
                for dk in range(DK):
                    st = w1t[:, dk, :] if j is None else w1t[:, j, dk, :]
                    nc.tensor.matmul(
                        ph[:, :cn],
                        st,
                        xt_sb[ci][:, dk, :],
                        start=(dk == 0),
                        stop=(dk == DK - 1),
                    )
                nc.scalar.activation(
                    h_sb[ci][:, ft, :],
                    ph[:, :cn],
                    mybir.ActivationFunctionType.Gelu,
                    bias=b1_sb[:, ft : ft + 1],
                    scale=1.0,
                )

            # Phase A. The first NWARM f-tiles run chunk 0 only, so the PE
            # has work while the rest of x is still in flight.
            order = [(ft, 0) for ft in range(NWARM)]
            order += [(ft, ci) for ci in range(1, NCH) for ft in range(NWARM)]
            order += [(ft, ci) for ft in range(NWARM, FT) for ci in range(NCH)]
            w2_done = 0
            for ft, ci in order:
                if ci == 0 and ft >= NSOLO and ft % 4 == 0:
                    g = (ft - NSOLO) // 4 + 2
                    if g < NG and g not in groups:
                        groups[g] = w1_group_dma(g)
                    # W2 prefetch rides the same ring behind the W1 stream.
                    if w2_done < 4:
                        nc.sync.dma_start(
                            out=w2_sb[:, :, w2_done, :], in_=w2_d[w2_done]
                        )
                        w2_done += 1
                gemm1_group(ft, ci)
            while w2_done < 4:
                nc.sync.dma_start(out=w2_sb[:, :, w2_done, :], in_=w2_d[w2_done])
                w2_done += 1

            # Phase B: token pairs, full 32-step PSUM accumulation per pair.
            for tq in range(TT // 2):
                ci = (tq * 2 * P) // 512  # chunk holding this token pair
                cb = tq * 2 * P - chunks[ci][0]  # base token within chunk
                accs = [
                    ypsum.tile([P, D], _F32, tag="py", name=f"py{tq}_{i}")
                    for i in range(2)
                ]
                for ft in range(FT):
                    for tt2 in range(2):
                        hblk = h_sb[ci][:, ft, cb + tt2 * P : cb + (tt2 + 1) * P]
                        for dh in range(2):
                            nc.tensor.matmul(
                                accs[tt2][:, dh * 512 : (dh + 1) * 512],
                                hblk,
                                w2_sb[:, ft // 4, ft % 4, dh * 512 : (dh + 1) * 512],
                                start=(ft == 0),
                                stop=(ft == FT - 1),
                            )
                ysb = ypool.tile([P, 2, D], _BF16, tag="ysb")
                for tt2 in range(2):
                    # Drain the two PSUM banks in parallel on Scalar+Vector.
                    nc.scalar.activation(
                        ysb[:, tt2, :512],
                        accs[tt2][:, :512],
                        mybir.ActivationFunctionType.Copy,
                    )
                    nc.vector.tensor_copy(ysb[:, tt2, 512:], accs[tt2][:, 512:])
                eng = nc.sync if tq % 2 == 0 else nc.gpsimd
                eng.dma_start(out=y_d[tq], in_=ysb[:])

    nc.compile()
    return nc


def _route(xf, Wr, br):
    """Host router: exact top-2 + softmax weights (float64 for stable order)."""
    logits = xf.astype(np.float64) @ Wr.astype(np.float64) + br.astype(np.float64)
    order = np.argsort(-logits, axis=1, kind="stable")
    top2 = order[:, :TOP_K]  # [T, 2]
    v = np.take_along_axis(logits, top2, axis=1)
    v = v - v.max(axis=1, keepdims=True)
    ev = np.exp(v)
    rw = (ev / ev.sum(axis=1, keepdims=True)).astype(np.float32)  # [T, 2]
    return top2, rw


def _run(x, Wr, br, W1, b1, W2, b2, trace=False):
    B, S, d = x.shape
    T = B * S
    xf = np.ascontiguousarray(np.asarray(x, dtype=np.float32).reshape(T, d))

    top2, rw = _route(xf, Wr, br)

    token_lists = []
    weight_lists = []
    for e in range(E):
        in_slot0 = top2[:, 0] == e
        in_slot1 = top2[:, 1] == e
        toks = np.nonzero(in_slot0 | in_slot1)[0]
        w = np.where(in_slot0[toks], rw[toks, 0], rw[toks, 1]).astype(np.float32)
        token_lists.append(toks)
        weight_lists.append(w)

    # Capacity: balanced mean (rounded up to 256), capped by the SBUF
    # working set (x + h + W2 are resident). Pairs beyond it are computed
    # on the host - cheap for near-balanced routing.
    C = max(256, min(1024, -(-(T * TOP_K // E) // 256) * 256))
    spill_lists = [(t[C:], w[C:]) for t, w in zip(token_lists, weight_lists)]
    token_lists = [t[:C] for t in token_lists]
    weight_lists = [w[:C] for w in weight_lists]

    if C not in _compiled:
        _compiled[C] = _build(C)
    nc = _compiled[C]

    # Per-expert weight layouts (see _build DRAM shapes)
    W1 = np.asarray(W1, dtype=np.float32)
    W2 = np.asarray(W2, dtype=np.float32)
    b1 = np.asarray(b1, dtype=np.float32)
    b2 = np.asarray(b2, dtype=np.float32)
    w1h = np.ascontiguousarray(
        W1.reshape(E, DK, P, FT, P).transpose(0, 3, 2, 1, 4)
    ).astype(BF16)  # [E, FT, P(dp), DK, P(fi)]
    w1a = np.ascontiguousarray(w1h[:, :NSOLO])  # [E, NSOLO, P, DK, P]
    NG = (FT - NSOLO) // 4
    w1g = np.ascontiguousarray(
        w1h[:, NSOLO:].reshape(E, NG, 4, P, DK, P).transpose(0, 1, 3, 2, 4, 5)
    )  # [E, NG, P, 4, DK, P]
    w2h = np.ascontiguousarray(
        W2.reshape(E, FT // 4, 4, P, D).transpose(0, 2, 3, 1, 4)
    ).astype(BF16)  # [E, 4(g), P(fp), FT//4, D]
    b1h = np.ascontiguousarray(b1.reshape(E, FT, P).transpose(0, 2, 1))  # [E, P, FT]

    in_maps = []
    for e in range(E):
        toks = token_lists[e]
        xg = np.zeros((C, d), dtype=np.float32)
        xg[: len(toks)] = xf[toks]
        xt = np.empty((P, DK * C), dtype=BF16)
        for c0, cn in _token_chunks(C):
            blk = xg[c0 : c0 + cn].T.reshape(DK, P, cn).transpose(1, 0, 2)
            xt[:, c0 * DK : c0 * DK + DK * cn] = blk.reshape(P, DK * cn).astype(BF16)
        in_maps.append(
            {"xt": xt, "w1a": w1a[e], "w1g": w1g[e], "w2": w2h[e], "b1": b1h[e]}
        )

    res = run_bass_kernel_spmd(
        nc, in_maps, core_ids=list(range(N_CORES)), trace=trace
    )

    # Host combine: out[t] = sum_k rw[t,k] * (y_{e_k}(t) + b2[e_k])
    w_dense = np.zeros((T, E), dtype=np.float32)
    np.put_along_axis(w_dense, top2, rw, axis=1)
    out = w_dense @ b2  # [T, D] bias part
    for e in range(E):
        toks = token_lists[e]
        yr = np.asarray(res.results[e]["y"], dtype=np.float32)  # [TT//2, P, 2, D]
        y = yr.transpose(0, 2, 1, 3).reshape(C, d)
        out[toks] += weight_lists[e][:, None] * y[: len(toks)]

    # Host-side spill: overflow pairs beyond the device capacity.
    try:
        from scipy.special import erf
    except ImportError:
        import math

        erf = np.vectorize(math.erf, otypes=[np.float32])

    sqrt2 = np.float32(np.sqrt(2.0))
    for e in range(E):
        toks, w = spill_lists[e]
        if len(toks) == 0:
            continue
        hs = xf[toks] @ W1[e] + b1[e]
        hs = 0.5 * hs * (1.0 + erf(hs / sqrt2))
        ys = hs @ W2[e]
        out[toks] += w[:, None] * ys

    return out.reshape(B, S, d).astype(np.float32), res


def kernel(x, Wr, br, W1, b1, W2, b2):
    out, _ = _run(x, Wr, br, W1, b1, W2, b2, trace=False)
    return out


# revision 32
# speedup vs baseline: 1.0820x; 1.0022x over previous
"""MoE (top-2 of 8 experts) Trainium2 kernel, expert-parallel across 8 NeuronCores.

Strategy (matches the expert-parallel sharding hint):
  - Host computes the router (logits -> top-2 -> softmax) and performs the
    token all-to-all: tokens are gathered per expert, padded to a common
    capacity C, and each core gets one expert's tokens + that expert's
    W1/b1/W2 weights.
  - Each core runs a Bass/Tile kernel computing
        y = gelu_exact(x @ W1 + b1) @ W2
    in bf16 (fp32 PSUM accumulate, ~3e-3 rel err, well under the 2e-2 gate).
  - Host scatter-adds the per-expert outputs back with the routing weights
    and adds sum_k w_k * b2[e_k] (folding b2 into the host combine).

Per-core dataflow (two phases, PE never idles between them):
  Phase A (h = gelu(x @ W1 + b1)): stationary = W1 128x128 blocks streamed
  from HBM, moving = xT token chunks; PSUM [f, 512 tok]; exact GELU +
  per-partition bias b1 fused into one ScalarE activation per tile; h kept
  RESIDENT in SBUF as bf16. The first K=8 f-tiles run on 256-token x chunks
  (startup HBM bandwidth is heavily contended while all 8 cores pull their
  first bytes) with emission supply-matched to the contended bandwidth; the
  rest use a 512-wide duplicate of the head tokens so the 150ns LDWEIGHTS
  stays hidden under 216ns matmuls.
  Phase B (y = h @ W2): W2 fully resident in one SBUF tile (prefetched
  behind the W1 stream); stationary = h blocks [128 f, 128 tok], moving =
  W2 rows [128 f, 512 d]; each token pair's y accumulates over all 32
  f-tiles in dedicated PSUM banks, then drains (ScalarE+VectorE halves in
  parallel -> bf16 -> DMA) while the next pair accumulates; the last pair
  ships each half as soon as it drains.
"""

import numpy as np
import ml_dtypes

import concourse.bass as bass
import concourse.mybir as mybir
import concourse.tile as tile
from concourse import bacc
from concourse.bass_utils import run_bass_kernel_spmd

P = 128
D = 1024
F = 4096
E = 8
TOP_K = 2
DK = D // P   # 8 contraction tiles for GEMM1
FT = F // P   # 32 f tiles
N_CORES = 8

BF16 = ml_dtypes.bfloat16

_F32 = mybir.dt.float32
_BF16 = mybir.dt.bfloat16

_compiled = {}  # C -> Bacc program


def _token_chunks(C):
    """Split C into 512-token chunks (PSUM-bank-width moving dim)."""
    chunks = []
    off = 0
    while off < C:
        cn = min(512, C - off)
        chunks.append((off, cn))
        off += cn
    return chunks


def _build(C):
    assert C % 256 == 0
    TT = C // P   # token tiles for GEMM2
    blocks = _token_chunks(C)   # 512-token blocks: h layout / phase B
    K = min(8, FT)              # f-tiles that run on 256-token x chunks
    nc = bacc.Bacc(None, target_bir_lowering=False)

    # x layout: tokens 0..511 twice - first as two 256-token chunks (a, b:
    # the startup-critical path, HBM is contended while all 8 cores pull
    # their first bytes), then the 512-blocks >= 512, then a 512-contiguous
    # duplicate of tokens 0..511 for the steady-state N=512 matmuls.
    XDUP = DK * min(512, C)
    xt_d = nc.dram_tensor("xt", [P, DK * C + XDUP], _BF16, kind="ExternalInput")
    w1_d = nc.dram_tensor("w1", [FT, P, DK, P], _BF16, kind="ExternalInput")
    w2_d = nc.dram_tensor("w2", [FT, P, D], _BF16, kind="ExternalInput")
    b1_d = nc.dram_tensor("b1", [P, FT], _F32, kind="ExternalInput")
    y_d = nc.dram_tensor("y", [TT // 2, P, 2, D], _BF16, kind="ExternalOutput")

    with tile.TileContext(nc) as tc:
        with (
            tc.tile_pool(name="xpool", bufs=1) as xpool,
            tc.tile_pool(name="cpool", bufs=1) as cpool,
            tc.tile_pool(name="w1pool", bufs=1) as w1pool,
            tc.tile_pool(name="w2pool", bufs=1) as w2pool,
            tc.tile_pool(name="hpool", bufs=1) as hpool,
            tc.tile_pool(name="ypool", bufs=2) as ypool,
            tc.tile_pool(name="hpsum", bufs=2, space="PSUM") as hpsum,
            tc.tile_pool(name="ypsum", bufs=3, space="PSUM") as ypsum,
        ):
            # All inputs ride the sync DMA ring in demand order (the ring is
            # FIFO; emission order is preserved for these uniform triggers).
            # gate() pins a trigger behind the first x chunk via a WAW write
            # into its destination, so the list-scheduler cannot hoist it.
            def gate(dst_corner, src_tile):
                nc.vector.tensor_copy(dst_corner, src_tile[:, 0, 0:2])

            w1_live = 8 + 4  # first 8 held through the deferred chunks
            solo = {}

            def w1_dma(ft, gated=False, eng=None):
                t = w1pool.tile(
                    [P, DK, P], _BF16, tag="w1t", bufs=w1_live, name=f"w1s{ft}"
                )
                if gated:
                    gate(t[:, 0, 0:2], xt_sb[0])
                (eng or nc.sync).dma_start(out=t[:], in_=w1_d[ft])
                solo[ft] = t

            w2_sb = w2pool.tile([P, FT, D], _BF16, name="w2sb")
            w2_fill = [0]

            def w2_dma(gated=False):
                k = w2_fill[0]
                if k < FT:
                    if gated:
                        gate(w2_sb[:, k, 0:2], xt_sb[0])
                    nc.sync.dma_start(out=w2_sb[:, k], in_=w2_d[k])
                    w2_fill[0] = k + 1

            h_sb = [
                hpool.tile([P, FT, cn], _BF16, tag=f"hc{ci}", name=f"hc{ci}")
                for ci, (_, cn) in enumerate(blocks)
            ]

            # chunk list for GEMM1: (x-dram-offset, width, h-offset)
            xchunks = [(0, 256, 0)]
            if C > 256:
                xchunks.append((DK * 256, 256, 256))
            for c0, cn in blocks[1:]:
                xchunks.append((DK * c0, cn, c0))
            full = None
            if C >= 512:
                full = len(xchunks)
                xchunks.append((DK * C, 512, 0))  # duplicate of tokens 0-511
            xt_sb = []

            def x_dma(si, gated=False):
                o, cn, _ = xchunks[si]
                t = xpool.tile([P, DK, cn], _BF16, tag=f"xt{si}", name=f"xt{si}")
                if gated:
                    gate(t[:, 0, 0:2], xt_sb[0])
                nc.sync.dma_start(out=t[:], in_=xt_d[:, o : o + DK * cn])
                xt_sb.append(t)

            # Startup emission, supply-matched to phase A consumption. The
            # startup-critical transfers split across both DMA rings so the
            # ~0.6us-per-trigger queue issue cost doesn't serialize them.
            x_dma(0)
            w1_dma(0, eng=nc.gpsimd)
            b1_sb = cpool.tile([P, FT], _F32)
            nc.gpsimd.dma_start(out=b1_sb[:], in_=b1_d[:])
            if C > 256:
                x_dma(1)
            w1_dma(1, eng=nc.gpsimd)
            for ft in range(2, 4):
                w1_dma(ft, eng=nc.gpsimd)
            for ft in range(4, K):
                w1_dma(ft, gated=True, eng=nc.gpsimd)
            for si in range(2, len(xchunks)):
                x_dma(si, gated=True)
            for ft in range(K, K + 2):
                if ft < FT:
                    w1_dma(ft, gated=True)

            # PE warm-up: dummy zero matmuls with no DMA deps run during the
            # initial input-DMA wait, so the HAM clock gate reaches 2.4 GHz
            # before the real stream starts. Memset on the idle VectorE.
            warm = cpool.tile([P, 512], _BF16, tag="warm")
            nc.vector.memset(warm[:], 0.0)
            for r in range(2):
                pw = hpsum.tile([P, 512], _F32, tag="ph", name=f"pw{r}")
                for k in range(6):
                    nc.tensor.matmul(
                        pw[:], warm[:, :P], warm[:], start=(k == 0), stop=(k == 5)
                    )

            def gemm1_group(ft, si):
                _, cn, h0 = xchunks[si]
                ph = hpsum.tile([P, 512], _F32, tag="ph")
                for dk in range(DK):
                    nc.tensor.matmul(
                        ph[:, :cn],
                        solo[ft][:, dk, :],
                        xt_sb[si][:, dk, :],
                        start=(dk == 0),
                        stop=(dk == DK - 1),
                    )
                nc.scalar.activation(
                    h_sb[h0 // 512][:, ft, h0 % 512 : h0 % 512 + cn],
                    ph[:, :cn],
                    mybir.ActivationFunctionType.Gelu,
                    bias=b1_sb[:, ft : ft + 1],
                    scale=1.0,
                )

            # Phase A order: the first K f-tiles run the 256-token chunks
            # while x streams in, then their deferred 512-blocks; the rest
            # run f-tile-major on the 512-wide duplicate.
            nab = 2 if C > 256 else 1
    		# chunk indices of the deferred 512-blocks (excluding the dup)
            later = list(range(nab, nab + len(blocks) - 1))
            order = [(ft, si) for ft in range(K) for si in range(nab)]
            order += [(ft, si) for si in later for ft in range(K)]
            rest = ([full] if full is not None else list(range(nab))) + later
            order += [(ft, si) for ft in range(FT) if ft >= K for si in rest]

            seen = set(ft for ft, _ in order[: K * nab])
            for ft, si in order:
                if ft not in solo:
                    w1_dma(ft, gated=True)
                if ft not in seen:
                    seen.add(ft)
                    # W1 lookahead + W2 prefetch ride the same ring.
                    la = ft + 2
                    if la < FT and la not in solo:
                        w1_dma(la, gated=True)
                    w2_dma(gated=w2_fill[0] < 2)
                    w2_dma(gated=w2_fill[0] < 2)
                gemm1_group(ft, si)
            while w2_fill[0] < FT:
                w2_dma()

            # Phase B: token pairs, full 32-step PSUM accumulation per pair.
            for tq in range(TT // 2):
                ci = (tq * 2 * P) // 512  # block holding this token pair
                cb = tq * 2 * P - blocks[ci][0]  # base token within block
                accs = [
                    ypsum.tile([P, D], _F32, tag="py", name=f"py{tq}_{i}")
                    for i in range(2)
                ]
                for ft in range(FT):
                    for tt2 in range(2):
                        hblk = h_sb[ci][:, ft, cb + tt2 * P : cb + (tt2 + 1) * P]
                        for dh in range(2):
                            nc.tensor.matmul(
                                accs[tt2][:, dh * 512 : (dh + 1) * 512],
                                hblk,
                                w2_sb[:, ft, dh * 512 : (dh + 1) * 512],
                                start=(ft == 0),
                                stop=(ft == FT - 1),
                            )
                ysb = ypool.tile([P, 2, D], _BF16, tag="ysb")
                eng = nc.sync if tq % 2 == 0 else nc.gpsimd
                last = tq == TT // 2 - 1
                for tt2 in range(2):
                    # Drain the two PSUM banks in parallel on Scalar+Vector.
                    nc.scalar.activation(
                        ysb[:, tt2, :512],
                        accs[tt2][:, :512],
                        mybir.ActivationFunctionType.Copy,
                    )
                    nc.vector.tensor_copy(ysb[:, tt2, 512:], accs[tt2][:, 512:])
                    if last:
                        # Ship each half as soon as it drains: the second
                        # half's DMA is the program's critical tail.
                        eng.dma_start(out=y_d[tq, :, tt2, :], in_=ysb[:, tt2, :])
                if not last:
                    eng.dma_start(out=y_d[tq], in_=ysb[:])

    nc.compile()
    return nc


def _route(xf, Wr, br):
    """Host router: exact top-2 + softmax weights (float64 for stable order)."""
    logits = xf.astype(np.float64) @ Wr.astype(np.float64) + br.astype(np.float64)
    order = np.argsort(-logits, axis=1, kind="stable")
    top2 = order[:, :TOP_K]  # [T, 2]
    v = np.take_along_axis(logits, top2, axis=1)
    v = v - v.max(axis=1, keepdims=True)
    ev = np.exp(v)
    rw = (ev / ev.sum(axis=1, keepdims=True)).astype(np.float32)  # [T, 2]
    return top2, rw


def _run(x, Wr, br, W1, b1, W2, b2, trace=False):
    B, S, d = x.shape
    T = B * S
    xf = np.ascontiguousarray(np.asarray(x, dtype=np.float32).reshape(T, d))

    top2, rw = _route(xf, Wr, br)

    token_lists = []
    weight_lists = []
    for e in range(E):
        in_slot0 = top2[:, 0] == e
        in_slot1 = top2[:, 1] == e
        toks = np.nonzero(in_slot0 | in_slot1)[0]
        w = np.where(in_slot0[toks], rw[toks, 0], rw[toks, 1]).astype(np.float32)
        token_lists.append(toks)
        weight_lists.append(w)

    # Capacity: balanced mean (rounded up to 256), capped by the SBUF
    # working set (x + h + W2 are resident). Pairs beyond it are computed
    # on the host - cheap for near-balanced routing.
    C = max(256, min(1024, -(-(T * TOP_K // E) // 256) * 256))
    spill_lists = [(t[C:], w[C:]) for t, w in zip(token_lists, weight_lists)]
    token_lists = [t[:C] for t in token_lists]
    weight_lists = [w[:C] for w in weight_lists]

    if C not in _compiled:
        _compiled[C] = _build(C)
    nc = _compiled[C]

    # Per-expert weight layouts (see _build DRAM shapes)
    W1 = np.asarray(W1, dtype=np.float32)
    W2 = np.asarray(W2, dtype=np.float32)
    b1 = np.asarray(b1, dtype=np.float32)
    b2 = np.asarray(b2, dtype=np.float32)
    w1h = np.ascontiguousarray(
        W1.reshape(E, DK, P, FT, P).transpose(0, 3, 2, 1, 4)
    ).astype(BF16)  # [E, FT, P(dp), DK, P(fi)]
    w2h = np.ascontiguousarray(W2.reshape(E, FT, P, D)).astype(BF16)  # [E, FT, P, D]
    b1h = np.ascontiguousarray(b1.reshape(E, FT, P).transpose(0, 2, 1))  # [E, P, FT]

    def pack(xg, c0, cn):
        blk = xg[c0 : c0 + cn].T.reshape(DK, P, cn).transpose(1, 0, 2)
        return blk.reshape(P, DK * cn).astype(BF16)

    XDUP = DK * min(512, C)
    in_maps = []
    for e in range(E):
        toks = token_lists[e]
        xg = np.zeros((C, d), dtype=np.float32)
        xg[: len(toks)] = xf[toks]
        xt = np.empty((P, DK * C + XDUP), dtype=BF16)
        chs = [(0, 256)] + ([(256, 256)] if C > 256 else [])
        chs += [(o, n) for o, n in _token_chunks(C) if o >= 512]
        for c0, cn in chs:
            xt[:, c0 * DK : c0 * DK + DK * cn] = pack(xg, c0, cn)
        # 512-contiguous duplicate of tokens 0..511 for steady-state N=512
        xt[:, DK * C :] = pack(xg, 0, min(512, C))
        in_maps.append({"xt": xt, "w1": w1h[e], "w2": w2h[e], "b1": b1h[e]})

    res = run_bass_kernel_spmd(
        nc, in_maps, core_ids=list(range(N_CORES)), trace=trace
    )

    # Host combine: out[t] = sum_k rw[t,k] * (y_{e_k}(t) + b2[e_k])
    w_dense = np.zeros((T, E), dtype=np.float32)
    np.put_along_axis(w_dense, top2, rw, axis=1)
    out = w_dense @ b2  # [T, D] bias part
    for e in range(E):
        toks = token_lists[e]
        yr = np.asarray(res.results[e]["y"], dtype=np.float32)  # [TT//2, P, 2, D]
        y = yr.transpose(0, 2, 1, 3).reshape(C, d)
        out[toks] += weight_lists[e][:, None] * y[: len(toks)]

    # Host-side spill: overflow pairs beyond the device capacity.
    try:
        from scipy.special import erf
    except ImportError:
        import math

        erf = np.vectorize(math.erf, otypes=[np.float32])

    sqrt2 = np.float32(np.sqrt(2.0))
    for e in range(E):
        toks, w = spill_lists[e]
        if len(toks) == 0:
            continue
        hs = xf[toks] @ W1[e] + b1[e]
        hs = 0.5 * hs * (1.0 + erf(hs / sqrt2))
        ys = hs @ W2[e]
        out[toks] += w[:, None] * ys

    return out.reshape(B, S, d).astype(np.float32), res


def kernel(x, Wr, br, W1, b1, W2, b2):
    out, _ = _run(x, Wr, br, W1, b1, W2, b2, trace=False)
    return out
